# revision 5
# baseline (speedup 1.0000x reference)
"""DiffJPEG forward (16x3x512x512, quality=80) on 8 TRN2 NeuronCores.

Strategy: pure data-parallel over batch (2 images/core). Per core, the whole
JPEG pipeline runs on-chip as 4 PE matmul stages (b,a,b,a alternation — the
form-b stages feed data as the stationary operand, which transposes for free):

  S1 (form-b): G1   = X^T A^T          [col, DCTrow]   (row DCT)
  S2 (form-a): G2   = Sum_c' L[c,c'] G1_c'  = F^T      (col DCT + fused 255*W_ycc color mix
                                                        + rank-1 -1024 DC offset for Y)
  quant      : Q    = round(G2 * (1/q)) * q             (DVE/ACT/GPSIMD, magic-number round)
  S3 (form-b): G3   = Q^T-chain = tq M (per block)     [DCTrow, col]
  S4 (form-a): R    = Sum_c V[chan,c] M^T G3_c + 128/255 (fused inverse color, rank-1 bias)
  clip       : out  = clamp(R, 0, 1)

A = kron(I, M) is block-diagonal, so form-b stages use banded N=256 windows
(2 accumulating matmuls per window). All matmuls run in float32r (1 cyc/row
at N>=256). Level shifts / color biases collapse into DC-coefficient rank-1
matmuls; quality-dependent quant tables arrive as tiny per-core input tensors
(the reference's qfull split over flattened (b,c)<16 means luma/chroma choice
varies per core; global slice index = 6*core + local_slice).
"""

import numpy as np

import concourse.bass as bass
import concourse.mybir as mybir
import concourse.tile as tile
from concourse import bacc
from concourse.bass_utils import run_bass_kernel_spmd

N_CORES = 8
BS = 16
IMGS_PER_CORE = BS // N_CORES          # 2
SLICES = IMGS_PER_CORE * 3             # 6
MAGIC = np.float32(1.5 * 2.0**23)      # fp32 round-to-nearest-even at ulp=1

F32 = mybir.dt.float32
F32R = mybir.dt.float32r
COPY = mybir.ActivationFunctionType.Copy

_LUM = np.array([[16,11,10,16,24,40,51,61],[12,12,14,19,26,58,60,55],[14,13,16,24,40,57,69,56],[14,17,22,29,51,87,80,62],[18,22,37,56,68,109,103,77],[24,35,55,64,81,104,113,92],[49,64,78,87,103,121,120,101],[72,92,95,98,112,100,103,99]], np.float32)
_CHROM = np.array([[17,18,24,47,99,99,99,99],[18,21,26,66,99,99,99,99],[24,26,56,99,99,99,99,99],[47,66,99,99,99,99,99,99],[99,99,99,99,99,99,99,99],[99,99,99,99,99,99,99,99],[99,99,99,99,99,99,99,99],[99,99,99,99,99,99,99,99]], np.float32)
_WYCC = np.array([[0.299, 0.587, 0.114], [-0.1687, -0.3313, 0.5], [0.5, -0.4187, -0.0813]], np.float32)
# inverse color terms: out_chan <- sum of coef * rec_channel (y=0, cb=1, cr=2)
_S4TERMS = [
    [(0, 1.0), (2, 1.402)],                       # r
    [(0, 1.0), (1, -0.34414), (2, -0.71414)],     # g
    [(0, 1.0), (1, 1.772)],                       # b
]


def _dct_mat():
    k = np.arange(8)[:, None]
    n = np.arange(8)[None, :]
    norm = np.where(k == 0, np.sqrt(1.0 / 8.0), np.sqrt(2.0 / 8.0))
    return (norm * np.cos(np.pi / 8.0 * (n + 0.5) * k)).astype(np.float32)


def _qtables(quality):
    q = max(1, min(100, int(quality)))
    scale = 5000.0 / q if q < 50 else 200.0 - 2.0 * q
    tbs = np.stack([_LUM, _CHROM]) * np.float32(scale)
    return np.clip((tbs + 50.0) / 100.0, 1.0, 255.0).astype(np.float32)


def _r11(x):
    """Round fp32 to float32r's 11-bit stored mantissa (RNE)."""
    xi = np.ascontiguousarray(x, np.float32).view(np.int32)
    s = 12
    xi = (xi + ((1 << (s - 1)) - 1) + ((xi >> s) & 1)) & ~((1 << s) - 1)
    return xi.view(np.float32)


def _host_constants():
    M = _dct_mat()
    BD = np.kron(np.eye(16, dtype=np.float32), M)       # kron(I16, M)
    BDT = np.ascontiguousarray(BD.T)                    # kron(I16, M^T)
    z = np.zeros((128, 256), np.float32)

    s1rhs = BDT.copy()                                  # fp32 [128,128], exact

    s2w = np.zeros((9, 128, 128), np.float32)           # BDT * 255*W[c,cp]
    for c in range(3):
        for cp in range(3):
            s2w[3 * c + cp] = BDT * np.float32(255.0 * _WYCC[c, cp])

    s3rhs = np.zeros((2, 128, 256), np.float32)         # [BD|0], [0|BD]
    s3rhs[0, :, :128] = BD
    s3rhs[1, :, 128:] = BD

    s4w = []
    s4idx = {}
    for chan in range(3):
        for (csrc, coef) in _S4TERMS[chan]:
            s4idx[(chan, csrc)] = len(s4w)
            s4w.append(BD * np.float32(coef / 255.0))
    s4w = np.stack(s4w)                                 # [7,128,128]

    m128 = np.arange(128)
    dcu = (-1024.0 * (m128 % 8 == 0)).astype(np.float32)[None, :]   # [1,128]
    dcv = ((np.arange(512) % 8 == 0).astype(np.float32))[None, :]   # [1,512]
    onu = np.ones((1, 128), np.float32)
    onv = np.full((1, 512), 128.0 / 255.0, np.float32)

    return dict(
        s1rhs=s1rhs, s2w=s2w, s3rhs=_r11(s3rhs), s4w=_r11(s4w),
        s4idx=s4idx, dcu=dcu, dcv=dcv, onu=_r11(onu), onv=_r11(onv),
    )


def _quant_inputs(quality, core):
    """Per-core [6,128,8] reciprocal-q and q pattern tiles.

    Quant runs on G2 = F^T laid out [v (partition), u (free)]:
    pattern value at (p, j) = qt[u=j, v=p%8]."""
    qt = _qtables(quality)
    rq = np.zeros((SLICES, 128, 8), np.float32)
    qq = np.zeros((SLICES, 128, 8), np.float32)
    p = np.arange(128)
    for i in range(SLICES):
        g = 6 * core + i                      # global flattened (b,c) slice
        tab = qt[0] if g < BS else qt[1]
        qq[i] = tab[:, p % 8].T               # [128,8]: [p, j] = tab[j, p%8]
        rq[i] = (1.0 / tab.astype(np.float64))[:, p % 8].T.astype(np.float32)
    return rq, qq


def _trace():
    hc = _host_constants()
    nc = bacc.Bacc("TRN2", target_bir_lowering=False, debug=False)

    img_d = nc.dram_tensor("img", [SLICES, 512, 512], F32, kind="ExternalInput").ap()
    rq_d = nc.dram_tensor("rqpat", [SLICES, 128, 8], F32, kind="ExternalInput").ap()
    qq_d = nc.dram_tensor("qpat", [SLICES, 128, 8], F32, kind="ExternalInput").ap()
    cst = {}
    for name in ("s1rhs", "s2w", "s3rhs", "s4w", "dcu", "dcv", "onu", "onv"):
        a = hc[name]
        cst[name] = nc.dram_tensor(name, list(a.shape), F32, kind="ExternalInput").ap()
    out_d = nc.dram_tensor("out", [SLICES, 512, 512], F32, kind="ExternalOutput").ap()

    s4idx = hc["s4idx"]

    with tile.TileContext(nc) as tc:
        with (
            tc.tile_pool(name="wts", bufs=1) as wp,
            tc.tile_pool(name="img", bufs=2) as imp,
            tc.tile_pool(name="g1", bufs=1) as g1p,
            tc.tile_pool(name="qq", bufs=1) as qp,
            tc.tile_pool(name="g3", bufs=1) as g3p,
            tc.tile_pool(name="ost", bufs=4) as op,
            tc.tile_pool(name="scr", bufs=3) as sp,
            tc.tile_pool(name="psA", bufs=4, space="PSUM") as psAp,
            tc.tile_pool(name="psB", bufs=4, space="PSUM") as psBp,
        ):
            # ---- constants into SBUF (f32r via SWDGE cast-DMA) ----
            s1r = wp.tile([128, 128], F32, tag="s1r")
            nc.sync.dma_start(s1r[:], cst["s1rhs"])
            s2w = wp.tile([128, 9 * 128], F32, tag="s2w")
            nc.sync.dma_start(s2w[:].rearrange("p (w n) -> p w n", w=9), cst["s2w"].rearrange("w p n -> p w n"))
            s3r = wp.tile([128, 512], F32R, tag="s3r")
            nc.gpsimd.dma_start(s3r[:].rearrange("p (w n) -> p w n", w=2), cst["s3rhs"].rearrange("w p n -> p w n"))
            s4w = wp.tile([128, 7 * 128], F32R, tag="s4w")
            nc.gpsimd.dma_start(s4w[:].rearrange("p (w n) -> p w n", w=7), cst["s4w"].rearrange("w p n -> p w n"))
            dcu = wp.tile([1, 128], F32, tag="dcu")
            nc.sync.dma_start(dcu[:], cst["dcu"])
            dcv = wp.tile([1, 512], F32, tag="dcv")
            nc.sync.dma_start(dcv[:], cst["dcv"])
            onu = wp.tile([1, 128], F32R, tag="onu")
            nc.gpsimd.dma_start(onu[:], cst["onu"])
            onv = wp.tile([1, 512], F32R, tag="onv")
            nc.gpsimd.dma_start(onv[:], cst["onv"])
            rqt = wp.tile([128, SLICES * 8], F32, tag="rqt")
            nc.sync.dma_start(rqt[:].rearrange("p (i j) -> p i j", j=8), rq_d.rearrange("i p j -> p i j"))
            qqt = wp.tile([128, SLICES * 8], F32, tag="qqt")
            nc.sync.dma_start(qqt[:].rearrange("p (i j) -> p i j", j=8), qq_d.rearrange("i p j -> p i j"))

            for im in range(IMGS_PER_CORE):
                # ---- image load: per channel [128, 2048] = (row%128, (slab, col)) ----
                xt = []
                for c in range(3):
                    t = imp.tile([128, 2048], F32, tag=f"x{c}")
                    nc.sync.dma_start(
                        t[:].rearrange("p (s c) -> p s c", s=4),
                        img_d[3 * im + c].rearrange("(s p) c -> p s c", p=128),
                    )
                    xt.append(t)

                # ---- S1: G1_c = X_c^T A^T  [col, DCTrow] ----
                g1 = []
                for c in range(3):
                    g = g1p.tile([128, 2048], F32, tag=f"g1_{c}")
                    g1.append(g)
                    for mt in range(4):
                        ps = psAp.tile([128, 512], F32, tag="psA")
                        for w in range(4):
                            nc.tensor.matmul(
                                ps[:, 128 * w : 128 * w + 128],
                                xt[c][:, 512 * w + 128 * mt : 512 * w + 128 * mt + 128],
                                s1r[:],
                                start=True, stop=True,
                            )
                        nc.scalar.activation(g[:, 512 * mt : 512 * mt + 512], ps[:], COPY)

                # ---- S2 + quant: G2_c = sum_cp L[c,cp] G1_cp (+DC), Q_c = dequant(round(G2*rq)) ----
                qt_ = []
                for c in range(3):
                    q = qp.tile([128, 2048], F32R, tag=f"q_{c}")
                    qt_.append(q)
                    sl = 3 * im + c
                    rqv = rqt[:, 8 * sl : 8 * sl + 8].rearrange("p (o j) -> p o j", o=1).broadcast_to((128, 64, 8))
                    qqv = qqt[:, 8 * sl : 8 * sl + 8].rearrange("p (o j) -> p o j", o=1).broadcast_to((128, 64, 8))
                    for s in range(4):
                        ps = psBp.tile([128, 512], F32, tag="psB")
                        for cp in range(3):
                            nc.tensor.matmul(
                                ps[:], s2w[:, 128 * (3 * c + cp) : 128 * (3 * c + cp) + 128],
                                g1[cp][:, 512 * s : 512 * s + 512],
                                start=(cp == 0), stop=(cp == 2 and c != 0),
                            )
                        if c == 0:
                            nc.tensor.matmul(ps[:], dcu[:], dcv[:], start=False, stop=True)
                        tb = sp.tile([128, 512], F32, tag="tq")
                        nc.vector.tensor_tensor(
                            tb[:].rearrange("p (a j) -> p a j", j=8),
                            ps[:].rearrange("p (a j) -> p a j", j=8),
                            rqv, op=mybir.AluOpType.mult,
                        )
                        nc.vector.tensor_scalar_add(tb[:], tb[:], float(MAGIC))
                        nc.vector.scalar_tensor_tensor(
                            q[:, 512 * s : 512 * s + 512].rearrange("p (a j) -> p a j", j=8),
                            tb[:].rearrange("p (a j) -> p a j", j=8),
                            float(MAGIC),
                            qqv,
                            op0=mybir.AluOpType.subtract,
                            op1=mybir.AluOpType.mult,
                        )

                # ---- S3: G3_c = (tq M-part)  [DCTrow, col] ----
                g3 = []
                for c in range(3):
                    g = g3p.tile([128, 2048], F32R, tag=f"g3_{c}")
                    g3.append(g)
                    for mt in range(4):
                        ps = psAp.tile([128, 512], F32, tag="psA")
                        for w in range(2):
                            for cc in range(2):
                                c2 = 2 * w + cc
                                nc.tensor.matmul(
                                    ps[:, 256 * w : 256 * w + 256],
                                    qt_[c][:, 512 * c2 + 128 * mt : 512 * c2 + 128 * mt + 128],
                                    s3r[:, 256 * cc : 256 * cc + 256],
                                    start=(cc == 0), stop=(cc == 1),
                                )
                        nc.scalar.activation(g[:, 512 * mt : 512 * mt + 512], ps[:], COPY)

                # ---- S4 + clip: out_chan = clamp(sum V M^T G3 + 128/255, 0, 1) ----
                for chan in range(3):
                    terms = _S4TERMS[chan]
                    for s in range(4):
                        ps = psBp.tile([128, 512], F32, tag="psB")
                        for ti, (csrc, _) in enumerate(terms):
                            wi = s4idx[(chan, csrc)]
                            nc.tensor.matmul(
                                ps[:], s4w[:, 128 * wi : 128 * wi + 128],
                                g3[csrc][:, 512 * s : 512 * s + 512],
                                start=(ti == 0), stop=False,
                            )
                        nc.tensor.matmul(ps[:], onu[:], onv[:], start=False, stop=True)
                        ot = op.tile([128, 512], F32, tag="ot")
                        nc.vector.tensor_scalar(
                            ot[:], ps[:], 0.0, 1.0,
                            op0=mybir.AluOpType.max, op1=mybir.AluOpType.min,
                        )
                        nc.sync.dma_start(
                            out_d[3 * im + chan, 128 * s : 128 * (s + 1), :], ot[:]
                        )

    nc.compile()
    return nc, hc


_COMPILED = None


def _get_compiled():
    global _COMPILED
    if _COMPILED is None:
        _COMPILED = _trace()
    return _COMPILED


def kernel(img, quality):
    img = np.ascontiguousarray(np.asarray(img, np.float32))
    quality = int(np.asarray(quality))
    nc, hc = _get_compiled()

    in_maps = []
    for core in range(N_CORES):
        rq, qq = _quant_inputs(quality, core)
        shard = np.ascontiguousarray(
            img[IMGS_PER_CORE * core : IMGS_PER_CORE * (core + 1)].reshape(SLICES, 512, 512)
        )
        in_maps.append({
            "img": shard, "rqpat": rq, "qpat": qq,
            "s1rhs": hc["s1rhs"], "s2w": hc["s2w"], "s3rhs": hc["s3rhs"],
            "s4w": hc["s4w"], "dcu": hc["dcu"], "dcv": hc["dcv"],
            "onu": hc["onu"], "onv": hc["onv"],
        })

    res = run_bass_kernel_spmd(nc, in_maps, core_ids=list(range(N_CORES)))
    out = np.stack([res.results[c]["out"] for c in range(N_CORES)])
    return out.reshape(BS, 3, 512, 512)


if __name__ == "__main__":
    rng = np.random.default_rng(0)
    x = rng.random((BS, 3, 512, 512), dtype=np.float32)
    y = kernel(x, 80)
    print("kernel ran:", y.shape, y.dtype, float(y.min()), float(y.max()))


# revision 7
# speedup vs baseline: 1.1183x; 1.1183x over previous
"""DiffJPEG forward (16x3x512x512, quality=80) on 8 TRN2 NeuronCores.

Strategy: pure data-parallel over batch (2 images/core). Per core, the whole
JPEG pipeline runs on-chip as 4 PE matmul stages (b,a,b,a alternation — the
form-b stages feed data as the stationary operand, which transposes for free):

  S1 (form-b): G1   = X^T A^T          [col, DCTrow]   (row DCT)
  S2 (form-a): G2   = Sum_c' L[c,c'] G1_c'  = F^T      (col DCT + fused 255*W_ycc color mix
                                                        + rank-1 -1024 DC offset for Y)
  quant      : Q    = round(G2 * (1/q)) * q             (DVE/ACT/GPSIMD, magic-number round)
  S3 (form-b): G3   = Q^T-chain = tq M (per block)     [DCTrow, col]
  S4 (form-a): R    = Sum_c V[chan,c] M^T G3_c + 128/255 (fused inverse color, rank-1 bias)
  clip       : out  = clamp(R, 0, 1)

A = kron(I, M) is block-diagonal, so form-b stages use banded N=256 windows
(2 accumulating matmuls per window). All matmuls run in float32r (1 cyc/row
at N>=256). Level shifts / color biases collapse into DC-coefficient rank-1
matmuls; quality-dependent quant tables arrive as tiny per-core input tensors
(the reference's qfull split over flattened (b,c)<16 means luma/chroma choice
varies per core; global slice index = 6*core + local_slice).
"""

import numpy as np

import concourse.bass as bass
import concourse.mybir as mybir
import concourse.tile as tile
from concourse import bacc
from concourse.bass_utils import run_bass_kernel_spmd

N_CORES = 8
BS = 16
IMGS_PER_CORE = BS // N_CORES          # 2
SLICES = IMGS_PER_CORE * 3             # 6
MAGIC = np.float32(1.5 * 2.0**23)      # fp32 round-to-nearest-even at ulp=1

F32 = mybir.dt.float32
F32R = mybir.dt.float32r
COPY = mybir.ActivationFunctionType.Copy

_LUM = np.array([[16,11,10,16,24,40,51,61],[12,12,14,19,26,58,60,55],[14,13,16,24,40,57,69,56],[14,17,22,29,51,87,80,62],[18,22,37,56,68,109,103,77],[24,35,55,64,81,104,113,92],[49,64,78,87,103,121,120,101],[72,92,95,98,112,100,103,99]], np.float32)
_CHROM = np.array([[17,18,24,47,99,99,99,99],[18,21,26,66,99,99,99,99],[24,26,56,99,99,99,99,99],[47,66,99,99,99,99,99,99],[99,99,99,99,99,99,99,99],[99,99,99,99,99,99,99,99],[99,99,99,99,99,99,99,99],[99,99,99,99,99,99,99,99]], np.float32)
_WYCC = np.array([[0.299, 0.587, 0.114], [-0.1687, -0.3313, 0.5], [0.5, -0.4187, -0.0813]], np.float32)
# inverse color terms: out_chan <- sum of coef * rec_channel (y=0, cb=1, cr=2)
_S4TERMS = [
    [(0, 1.0), (2, 1.402)],                       # r
    [(0, 1.0), (1, -0.34414), (2, -0.71414)],     # g
    [(0, 1.0), (1, 1.772)],                       # b
]


def _dct_mat():
    k = np.arange(8)[:, None]
    n = np.arange(8)[None, :]
    norm = np.where(k == 0, np.sqrt(1.0 / 8.0), np.sqrt(2.0 / 8.0))
    return (norm * np.cos(np.pi / 8.0 * (n + 0.5) * k)).astype(np.float32)


def _qtables(quality):
    q = max(1, min(100, int(quality)))
    scale = 5000.0 / q if q < 50 else 200.0 - 2.0 * q
    tbs = np.stack([_LUM, _CHROM]) * np.float32(scale)
    return np.clip((tbs + 50.0) / 100.0, 1.0, 255.0).astype(np.float32)


def _r11(x):
    """Round fp32 to float32r's 11-bit stored mantissa (RNE)."""
    xi = np.ascontiguousarray(x, np.float32).view(np.int32)
    s = 12
    xi = (xi + ((1 << (s - 1)) - 1) + ((xi >> s) & 1)) & ~((1 << s) - 1)
    return xi.view(np.float32)


def _host_constants():
    M = _dct_mat()
    BD = np.kron(np.eye(16, dtype=np.float32), M)       # kron(I16, M)
    BDT = np.ascontiguousarray(BD.T)                    # kron(I16, M^T)
    z = np.zeros((128, 256), np.float32)

    s1rhs = BDT.copy()                                  # fp32 [128,128], exact

    s2w = np.zeros((9, 128, 128), np.float32)           # BDT * 255*W[c,cp]
    for c in range(3):
        for cp in range(3):
            s2w[3 * c + cp] = BDT * np.float32(255.0 * _WYCC[c, cp])

    s3rhs = np.zeros((2, 128, 256), np.float32)         # [BD|0], [0|BD]
    s3rhs[0, :, :128] = BD
    s3rhs[1, :, 128:] = BD

    s4w = []
    s4idx = {}
    for chan in range(3):
        for (csrc, coef) in _S4TERMS[chan]:
            s4idx[(chan, csrc)] = len(s4w)
            s4w.append(BD * np.float32(coef / 255.0))
    s4w = np.stack(s4w)                                 # [7,128,128]

    m128 = np.arange(128)
    dcu = (-1024.0 * (m128 % 8 == 0)).astype(np.float32)[None, :]   # [1,128]
    dcv = ((np.arange(512) % 8 == 0).astype(np.float32))[None, :]   # [1,512]
    onu = np.ones((1, 128), np.float32)
    onv = np.full((1, 512), 128.0 / 255.0, np.float32)

    s2wh = _r11(s2w)
    s2wl = s2w - s2wh          # exact 12-bit residual, f32r-representable
    return dict(
        s1rhs=s1rhs, s2wh=s2wh, s2wl=s2wl, s3rhs=_r11(s3rhs), s4w=_r11(s4w),
        s4idx=s4idx, dcu=_r11(dcu), dcv=_r11(dcv), onu=_r11(onu), onv=_r11(onv),
    )


def _quant_inputs(quality, core):
    """Per-core [6,128,8] reciprocal-q and q pattern tiles.

    Quant runs on G2 = F^T laid out [v (partition), u (free)]:
    pattern value at (p, j) = qt[u=j, v=p%8]."""
    qt = _qtables(quality)
    rq = np.zeros((SLICES, 128, 8), np.float32)
    qq = np.zeros((SLICES, 128, 8), np.float32)
    p = np.arange(128)
    for i in range(SLICES):
        g = 6 * core + i                      # global flattened (b,c) slice
        tab = qt[0] if g < BS else qt[1]
        qq[i] = tab[:, p % 8].T               # [128,8]: [p, j] = tab[j, p%8]
        rq[i] = (1.0 / tab.astype(np.float64))[:, p % 8].T.astype(np.float32)
    return rq, qq


def _trace():
    hc = _host_constants()
    nc = bacc.Bacc("TRN2", target_bir_lowering=False, debug=False)

    img_d = nc.dram_tensor("img", [SLICES, 512, 512], F32, kind="ExternalInput").ap()
    rq_d = nc.dram_tensor("rqpat", [SLICES, 128, 8], F32, kind="ExternalInput").ap()
    qq_d = nc.dram_tensor("qpat", [SLICES, 128, 8], F32, kind="ExternalInput").ap()
    cst = {}
    for name in ("s1rhs", "s2wh", "s2wl", "s3rhs", "s4w", "dcu", "dcv", "onu", "onv"):
        a = hc[name]
        cst[name] = nc.dram_tensor(name, list(a.shape), F32, kind="ExternalInput").ap()
    out_d = nc.dram_tensor("out", [SLICES, 512, 512], F32, kind="ExternalOutput").ap()

    s4idx = hc["s4idx"]

    with tile.TileContext(nc) as tc:
        with (
            tc.tile_pool(name="wts", bufs=1) as wp,
            tc.tile_pool(name="img", bufs=2) as imp,
            tc.tile_pool(name="g1", bufs=1) as g1p,
            tc.tile_pool(name="qq", bufs=1) as qp,
            tc.tile_pool(name="g3", bufs=1) as g3p,
            tc.tile_pool(name="ost", bufs=4) as op,
            tc.tile_pool(name="scr", bufs=3) as sp,
            tc.tile_pool(name="psA", bufs=4, space="PSUM") as psAp,
            tc.tile_pool(name="psB", bufs=4, space="PSUM") as psBp,
        ):
            # ---- constants into SBUF (f32r via SWDGE cast-DMA) ----
            s1r = wp.tile([128, 128], F32, tag="s1r")
            nc.sync.dma_start(s1r[:], cst["s1rhs"])
            s2wh = wp.tile([128, 9 * 128], F32R, tag="s2wh")
            nc.gpsimd.dma_start(s2wh[:].rearrange("p (w n) -> p w n", w=9), cst["s2wh"].rearrange("w p n -> p w n"))
            s2wl = wp.tile([128, 9 * 128], F32R, tag="s2wl")
            nc.gpsimd.dma_start(s2wl[:].rearrange("p (w n) -> p w n", w=9), cst["s2wl"].rearrange("w p n -> p w n"))
            s3r = wp.tile([128, 512], F32R, tag="s3r")
            nc.gpsimd.dma_start(s3r[:].rearrange("p (w n) -> p w n", w=2), cst["s3rhs"].rearrange("w p n -> p w n"))
            s4w = wp.tile([128, 7 * 128], F32R, tag="s4w")
            nc.gpsimd.dma_start(s4w[:].rearrange("p (w n) -> p w n", w=7), cst["s4w"].rearrange("w p n -> p w n"))
            dcu = wp.tile([1, 128], F32R, tag="dcu")
            nc.gpsimd.dma_start(dcu[:], cst["dcu"])
            dcv = wp.tile([1, 512], F32R, tag="dcv")
            nc.gpsimd.dma_start(dcv[:], cst["dcv"])
            onu = wp.tile([1, 128], F32R, tag="onu")
            nc.gpsimd.dma_start(onu[:], cst["onu"])
            onv = wp.tile([1, 512], F32R, tag="onv")
            nc.gpsimd.dma_start(onv[:], cst["onv"])
            rqt = wp.tile([128, SLICES * 8], F32, tag="rqt")
            nc.sync.dma_start(rqt[:].rearrange("p (i j) -> p i j", j=8), rq_d.rearrange("i p j -> p i j"))
            qqt = wp.tile([128, SLICES * 8], F32, tag="qqt")
            nc.sync.dma_start(qqt[:].rearrange("p (i j) -> p i j", j=8), qq_d.rearrange("i p j -> p i j"))

            for im in range(IMGS_PER_CORE):
                # ---- image load: per channel [128, 2048] = (row%128, (slab, col)) ----
                xt = []
                for c in range(3):
                    t = imp.tile([128, 2048], F32, tag=f"x{c}")
                    nc.sync.dma_start(
                        t[:].rearrange("p (s c) -> p s c", s=4),
                        img_d[3 * im + c].rearrange("(s p) c -> p s c", p=128),
                    )
                    xt.append(t)

                # ---- S1: G1_c = X_c^T A^T  [col, DCTrow] ----
                g1 = []
                g1l = []
                for c in range(3):
                    g = g1p.tile([128, 2048], F32R, tag=f"g1_{c}")
                    gl = g1p.tile([128, 2048], F32R, tag=f"g1l_{c}")
                    g1.append(g)
                    g1l.append(gl)
                    for mt in range(4):
                        ps = psAp.tile([128, 512], F32, tag="psA")
                        for w in range(4):
                            nc.tensor.matmul(
                                ps[:, 128 * w : 128 * w + 128],
                                xt[c][:, 512 * w + 128 * mt : 512 * w + 128 * mt + 128],
                                s1r[:],
                                start=True, stop=True,
                            )
                        nc.scalar.activation(g[:, 512 * mt : 512 * mt + 512], ps[:], COPY)
                        nc.vector.scalar_tensor_tensor(
                            gl[:, 512 * mt : 512 * mt + 512], ps[:], 0.0,
                            g[:, 512 * mt : 512 * mt + 512],
                            op0=mybir.AluOpType.add, op1=mybir.AluOpType.subtract,
                        )

                # ---- S2 + quant: G2_c = sum_cp L[c,cp] G1_cp (+DC), Q_c = dequant(round(G2*rq)) ----
                qt_ = []
                for c in range(3):
                    q = qp.tile([128, 2048], F32R, tag=f"q_{c}")
                    qt_.append(q)
                    sl = 3 * im + c
                    rqv = rqt[:, 8 * sl : 8 * sl + 8].rearrange("p (o j) -> p o j", o=1).broadcast_to((128, 64, 8))
                    qqv = qqt[:, 8 * sl : 8 * sl + 8].rearrange("p (o j) -> p o j", o=1).broadcast_to((128, 64, 8))
                    for s in range(4):
                        ps = psBp.tile([128, 512], F32, tag="psB")
                        nmm = 9 + (1 if c == 0 else 0)
                        k = 0
                        for (wt, dat) in ((s2wh, g1), (s2wh, g1l), (s2wl, g1)):
                            for cp in range(3):
                                nc.tensor.matmul(
                                    ps[:], wt[:, 128 * (3 * c + cp) : 128 * (3 * c + cp) + 128],
                                    dat[cp][:, 512 * s : 512 * s + 512],
                                    start=(k == 0), stop=(k == nmm - 1),
                                )
                                k += 1
                        if c == 0:
                            nc.tensor.matmul(ps[:], dcu[:], dcv[:], start=False, stop=True)
                        tb = sp.tile([128, 512], F32, tag="tq")
                        nc.vector.tensor_tensor(
                            tb[:].rearrange("p (a j) -> p a j", j=8),
                            ps[:].rearrange("p (a j) -> p a j", j=8),
                            rqv, op=mybir.AluOpType.mult,
                        )
                        nc.scalar.activation(tb[:], tb[:], COPY, bias=float(MAGIC))
                        nc.vector.scalar_tensor_tensor(
                            q[:, 512 * s : 512 * s + 512].rearrange("p (a j) -> p a j", j=8),
                            tb[:].rearrange("p (a j) -> p a j", j=8),
                            float(MAGIC),
                            qqv,
                            op0=mybir.AluOpType.subtract,
                            op1=mybir.AluOpType.mult,
                        )

                # ---- S3: G3_c = (tq M-part)  [DCTrow, col] ----
                g3 = []
                for c in range(3):
                    g = g3p.tile([128, 2048], F32R, tag=f"g3_{c}")
                    g3.append(g)
                    for mt in range(4):
                        ps = psAp.tile([128, 512], F32, tag="psA")
                        for w in range(2):
                            for cc in range(2):
                                c2 = 2 * w + cc
                                nc.tensor.matmul(
                                    ps[:, 256 * w : 256 * w + 256],
                                    qt_[c][:, 512 * c2 + 128 * mt : 512 * c2 + 128 * mt + 128],
                                    s3r[:, 256 * cc : 256 * cc + 256],
                                    start=(cc == 0), stop=(cc == 1),
                                )
                        nc.scalar.activation(g[:, 512 * mt : 512 * mt + 512], ps[:], COPY)
                        nc.vector.scalar_tensor_tensor(
                            gl[:, 512 * mt : 512 * mt + 512], ps[:], 0.0,
                            g[:, 512 * mt : 512 * mt + 512],
                            op0=mybir.AluOpType.add, op1=mybir.AluOpType.subtract,
                        )

                # ---- S4 + clip: out_chan = clamp(sum V M^T G3 + 128/255, 0, 1) ----
                for chan in range(3):
                    terms = _S4TERMS[chan]
                    for s in range(4):
                        ps = psBp.tile([128, 512], F32, tag="psB")
                        for ti, (csrc, _) in enumerate(terms):
                            wi = s4idx[(chan, csrc)]
                            nc.tensor.matmul(
                                ps[:], s4w[:, 128 * wi : 128 * wi + 128],
                                g3[csrc][:, 512 * s : 512 * s + 512],
                                start=(ti == 0), stop=False,
                            )
                        nc.tensor.matmul(ps[:], onu[:], onv[:], start=False, stop=True)
                        ot = op.tile([128, 512], F32, tag="ot")
                        nc.vector.tensor_scalar(
                            ot[:], ps[:], 0.0, 1.0,
                            op0=mybir.AluOpType.max, op1=mybir.AluOpType.min,
                        )
                        nc.sync.dma_start(
                            out_d[3 * im + chan, 128 * s : 128 * (s + 1), :], ot[:]
                        )

    nc.compile()
    return nc, hc


_COMPILED = None


def _get_compiled():
    global _COMPILED
    if _COMPILED is None:
        _COMPILED = _trace()
    return _COMPILED


def kernel(img, quality):
    img = np.ascontiguousarray(np.asarray(img, np.float32))
    quality = int(np.asarray(quality))
    nc, hc = _get_compiled()

    in_maps = []
    for core in range(N_CORES):
        rq, qq = _quant_inputs(quality, core)
        shard = np.ascontiguousarray(
            img[IMGS_PER_CORE * core : IMGS_PER_CORE * (core + 1)].reshape(SLICES, 512, 512)
        )
        in_maps.append({
            "img": shard, "rqpat": rq, "qpat": qq,
            "s1rhs": hc["s1rhs"], "s2wh": hc["s2wh"], "s2wl": hc["s2wl"], "s3rhs": hc["s3rhs"],
            "s4w": hc["s4w"], "dcu": hc["dcu"], "dcv": hc["dcv"],
            "onu": hc["onu"], "onv": hc["onv"],
        })

    res = run_bass_kernel_spmd(nc, in_maps, core_ids=list(range(N_CORES)))
    out = np.stack([res.results[c]["out"] for c in range(N_CORES)])
    return out.reshape(BS, 3, 512, 512)


if __name__ == "__main__":
    rng = np.random.default_rng(0)
    x = rng.random((BS, 3, 512, 512), dtype=np.float32)
    y = kernel(x, 80)
    print("kernel ran:", y.shape, y.dtype, float(y.min()), float(y.max()))


# revision 10
# speedup vs baseline: 1.1567x; 1.0344x over previous
"""DiffJPEG forward (16x3x512x512, quality=80) on 8 TRN2 NeuronCores.

Strategy: pure data-parallel over batch (2 images/core). Per core, the whole
JPEG pipeline runs on-chip as 4 PE matmul stages (b,a,b,a alternation — the
form-b stages feed data as the stationary operand, which transposes for free):

  S1 (form-b): G1   = X^T A^T          [col, DCTrow]   (row DCT)
  S2 (form-a): G2   = Sum_c' L[c,c'] G1_c'  = F^T      (col DCT + fused 255*W_ycc color mix
                                                        + rank-1 -1024 DC offset for Y)
  quant      : Q    = round(G2 * (1/q)) * q             (DVE/ACT/GPSIMD, magic-number round)
  S3 (form-b): G3   = Q^T-chain = tq M (per block)     [DCTrow, col]
  S4 (form-a): R    = Sum_c V[chan,c] M^T G3_c + 128/255 (fused inverse color, rank-1 bias)
  clip       : out  = clamp(R, 0, 1)

A = kron(I, M) is block-diagonal, so form-b stages use banded N=256 windows
(2 accumulating matmuls per window). All matmuls run in float32r (1 cyc/row
at N>=256). Level shifts / color biases collapse into DC-coefficient rank-1
matmuls; quality-dependent quant tables arrive as tiny per-core input tensors
(the reference's qfull split over flattened (b,c)<16 means luma/chroma choice
varies per core; global slice index = 6*core + local_slice).
"""

import numpy as np

import concourse.bass as bass
import concourse.mybir as mybir
import concourse.tile as tile
from concourse import bacc
from concourse.bass_utils import run_bass_kernel_spmd

N_CORES = 8
BS = 16
IMGS_PER_CORE = BS // N_CORES          # 2
SLICES = IMGS_PER_CORE * 3             # 6
MAGIC = np.float32(1.5 * 2.0**23)      # fp32 round-to-nearest-even at ulp=1

F32 = mybir.dt.float32
F32R = mybir.dt.float32r
COPY = mybir.ActivationFunctionType.Copy

_LUM = np.array([[16,11,10,16,24,40,51,61],[12,12,14,19,26,58,60,55],[14,13,16,24,40,57,69,56],[14,17,22,29,51,87,80,62],[18,22,37,56,68,109,103,77],[24,35,55,64,81,104,113,92],[49,64,78,87,103,121,120,101],[72,92,95,98,112,100,103,99]], np.float32)
_CHROM = np.array([[17,18,24,47,99,99,99,99],[18,21,26,66,99,99,99,99],[24,26,56,99,99,99,99,99],[47,66,99,99,99,99,99,99],[99,99,99,99,99,99,99,99],[99,99,99,99,99,99,99,99],[99,99,99,99,99,99,99,99],[99,99,99,99,99,99,99,99]], np.float32)
_WYCC = np.array([[0.299, 0.587, 0.114], [-0.1687, -0.3313, 0.5], [0.5, -0.4187, -0.0813]], np.float32)
# inverse color terms: out_chan <- sum of coef * rec_channel (y=0, cb=1, cr=2)
_S4TERMS = [
    [(0, 1.0), (2, 1.402)],                       # r
    [(0, 1.0), (1, -0.34414), (2, -0.71414)],     # g
    [(0, 1.0), (1, 1.772)],                       # b
]


def _dct_mat():
    k = np.arange(8)[:, None]
    n = np.arange(8)[None, :]
    norm = np.where(k == 0, np.sqrt(1.0 / 8.0), np.sqrt(2.0 / 8.0))
    return (norm * np.cos(np.pi / 8.0 * (n + 0.5) * k)).astype(np.float32)


def _qtables(quality):
    q = max(1, min(100, int(quality)))
    scale = 5000.0 / q if q < 50 else 200.0 - 2.0 * q
    tbs = np.stack([_LUM, _CHROM]) * np.float32(scale)
    return np.clip((tbs + 50.0) / 100.0, 1.0, 255.0).astype(np.float32)


def _r11(x):
    """Round fp32 to float32r's 11-bit stored mantissa (RNE)."""
    xi = np.ascontiguousarray(x, np.float32).view(np.int32)
    s = 12
    xi = (xi + ((1 << (s - 1)) - 1) + ((xi >> s) & 1)) & ~((1 << s) - 1)
    return xi.view(np.float32)


def _host_constants():
    M = _dct_mat()
    BD = np.kron(np.eye(16, dtype=np.float32), M)       # kron(I16, M)
    BDT = np.ascontiguousarray(BD.T)                    # kron(I16, M^T)
    z = np.zeros((128, 256), np.float32)

    s1rhs = BDT.copy()                                  # fp32 [128,128], exact

    s2w = np.zeros((9, 128, 128), np.float32)           # BDT * 255*W[c,cp]
    for c in range(3):
        for cp in range(3):
            s2w[3 * c + cp] = BDT * np.float32(255.0 * _WYCC[c, cp])

    s3rhs = np.zeros((2, 128, 256), np.float32)         # [BD|0], [0|BD]
    s3rhs[0, :, :128] = BD
    s3rhs[1, :, 128:] = BD

    s4w = []
    s4idx = {}
    for chan in range(3):
        for (csrc, coef) in _S4TERMS[chan]:
            s4idx[(chan, csrc)] = len(s4w)
            s4w.append(BD * np.float32(coef / 255.0))
    s4w = np.stack(s4w)                                 # [7,128,128]

    m128 = np.arange(128)
    dcu = (-1024.0 * (m128 % 8 == 0)).astype(np.float32)[None, :]   # [1,128]
    dcv = ((np.arange(512) % 8 == 0).astype(np.float32))[None, :]   # [1,512]
    onu = np.ones((1, 128), np.float32)
    onv = np.full((1, 512), 128.0 / 255.0, np.float32)

    s2wh = _r11(s2w)
    s2wl = s2w - s2wh          # exact 12-bit residual, f32r-representable
    return dict(
        s1rhs=s1rhs, s2wh=s2wh, s2wl=s2wl, s3rhs=_r11(s3rhs), s4w=_r11(s4w),
        s4idx=s4idx, dcu=_r11(dcu), dcv=_r11(dcv), onu=_r11(onu), onv=_r11(onv),
    )


def _quant_inputs(quality, core):
    """Per-core [6,128,8] reciprocal-q and q pattern tiles.

    Quant runs on G2 = F^T laid out [v (partition), u (free)]:
    pattern value at (p, j) = qt[u=j, v=p%8]."""
    qt = _qtables(quality)
    rq = np.zeros((SLICES, 128, 8), np.float32)
    qq = np.zeros((SLICES, 128, 8), np.float32)
    p = np.arange(128)
    for i in range(SLICES):
        g = 6 * core + i                      # global flattened (b,c) slice
        tab = qt[0] if g < BS else qt[1]
        qq[i] = tab[:, p % 8].T               # [128,8]: [p, j] = tab[j, p%8]
        rq[i] = (1.0 / tab.astype(np.float64))[:, p % 8].T.astype(np.float32)
    return rq, qq


def _trace():
    hc = _host_constants()
    nc = bacc.Bacc("TRN2", target_bir_lowering=False, debug=False)

    img_d = nc.dram_tensor("img", [SLICES, 512, 512], F32, kind="ExternalInput").ap()
    rq_d = nc.dram_tensor("rqpat", [SLICES, 128, 8], F32, kind="ExternalInput").ap()
    qq_d = nc.dram_tensor("qpat", [SLICES, 128, 8], F32, kind="ExternalInput").ap()
    cst = {}
    for name in ("s1rhs", "s2wh", "s2wl", "s3rhs", "s4w", "dcu", "dcv", "onu", "onv"):
        a = hc[name]
        cst[name] = nc.dram_tensor(name, list(a.shape), F32, kind="ExternalInput").ap()
    out_d = nc.dram_tensor("out", [SLICES, 512, 512], F32, kind="ExternalOutput").ap()

    s4idx = hc["s4idx"]

    with tile.TileContext(nc) as tc:
        with (
            tc.tile_pool(name="wts", bufs=1) as wp,
            tc.tile_pool(name="img", bufs=2) as imp,
            tc.tile_pool(name="g1", bufs=1) as g1p,
            tc.tile_pool(name="qq", bufs=1) as qp,
            tc.tile_pool(name="g3", bufs=1) as g3p,
            tc.tile_pool(name="ost", bufs=4) as op,
            tc.tile_pool(name="scr", bufs=3) as sp,
            tc.tile_pool(name="psA", bufs=4, space="PSUM") as psAp,
            tc.tile_pool(name="psB", bufs=4, space="PSUM") as psBp,
        ):
            # ---- constants into SBUF (f32r via SWDGE cast-DMA) ----
            s1r = wp.tile([128, 128], F32, tag="s1r")
            nc.sync.dma_start(s1r[:], cst["s1rhs"])
            early_imgs = []
            for _c in range(3):
                _t = imp.tile([128, 2048], F32, tag=f"x{_c}")
                nc.sync.dma_start(
                    _t[:].rearrange("p (s c) -> p s c", s=4),
                    img_d[_c].rearrange("(s p) c -> p s c", p=128),
                )
                early_imgs.append(_t)
            s2wh = wp.tile([128, 9 * 128], F32R, tag="s2wh")
            nc.gpsimd.dma_start(s2wh[:].rearrange("p (w n) -> p w n", w=9), cst["s2wh"].rearrange("w p n -> p w n"))
            s2wl = wp.tile([128, 9 * 128], F32R, tag="s2wl")
            nc.gpsimd.dma_start(s2wl[:].rearrange("p (w n) -> p w n", w=9), cst["s2wl"].rearrange("w p n -> p w n"))
            s3r = wp.tile([128, 512], F32R, tag="s3r")
            nc.gpsimd.dma_start(s3r[:].rearrange("p (w n) -> p w n", w=2), cst["s3rhs"].rearrange("w p n -> p w n"))
            s4w = wp.tile([128, 7 * 128], F32R, tag="s4w")
            nc.gpsimd.dma_start(s4w[:].rearrange("p (w n) -> p w n", w=7), cst["s4w"].rearrange("w p n -> p w n"))
            dcu = wp.tile([1, 128], F32R, tag="dcu")
            nc.gpsimd.dma_start(dcu[:], cst["dcu"])
            dcv = wp.tile([1, 512], F32R, tag="dcv")
            nc.gpsimd.dma_start(dcv[:], cst["dcv"])
            onu = wp.tile([1, 128], F32R, tag="onu")
            nc.gpsimd.dma_start(onu[:], cst["onu"])
            onv = wp.tile([1, 512], F32R, tag="onv")
            nc.gpsimd.dma_start(onv[:], cst["onv"])
            rqt = wp.tile([128, SLICES * 8], F32, tag="rqt")
            nc.sync.dma_start(rqt[:].rearrange("p (i j) -> p i j", j=8), rq_d.rearrange("i p j -> p i j"))
            qqt = wp.tile([128, SLICES * 8], F32, tag="qqt")
            nc.sync.dma_start(qqt[:].rearrange("p (i j) -> p i j", j=8), qq_d.rearrange("i p j -> p i j"))

            state = {}

            def s_load(im):
                if im == 0:
                    state[("x", 0)] = early_imgs
                    return
                xt = []
                for c in range(3):
                    t = imp.tile([128, 2048], F32, tag=f"x{c}")
                    nc.sync.dma_start(
                        t[:].rearrange("p (s c) -> p s c", s=4),
                        img_d[3 * im + c].rearrange("(s p) c -> p s c", p=128),
                    )
                    xt.append(t)
                state[("x", im)] = xt

            def s1(im):
                xt = state[("x", im)]
                g1, g1l = [], []
                for c in range(3):
                    g = g1p.tile([128, 2048], F32R, tag=f"g1_{c}")
                    gl = g1p.tile([128, 2048], F32R, tag=f"g1l_{c}")
                    g1.append(g)
                    g1l.append(gl)
                    for mt in range(4):
                        ps = psAp.tile([128, 512], F32, tag="psA")
                        for w in range(4):
                            nc.tensor.matmul(
                                ps[:, 128 * w : 128 * w + 128],
                                xt[c][:, 512 * w + 128 * mt : 512 * w + 128 * mt + 128],
                                s1r[:],
                                start=True, stop=True,
                            )
                        nc.scalar.activation(g[:, 512 * mt : 512 * mt + 512], ps[:], COPY)
                        nc.vector.scalar_tensor_tensor(
                            gl[:, 512 * mt : 512 * mt + 512], ps[:], 0.0,
                            g[:, 512 * mt : 512 * mt + 512],
                            op0=mybir.AluOpType.add, op1=mybir.AluOpType.subtract,
                        )
                state[("g1", im)] = (g1, g1l)

            def s2q(im, chans=(0, 1, 2)):
                g1, g1l = state[("g1", im)]
                qt_ = state.setdefault(("q", im), [None, None, None])
                for c in chans:
                    q = qp.tile([128, 2048], F32R, tag=f"q_{c}")
                    qt_[c] = q
                    sl = 3 * im + c
                    rqv = rqt[:, 8 * sl : 8 * sl + 8].rearrange("p (o j) -> p o j", o=1).broadcast_to((128, 64, 8))
                    qqv = qqt[:, 8 * sl : 8 * sl + 8].rearrange("p (o j) -> p o j", o=1).broadcast_to((128, 64, 8))
                    for s in range(4):
                        ps = psBp.tile([128, 512], F32, tag="psB")
                        nmm = 9 + (1 if c == 0 else 0)
                        k = 0
                        for (wt, dat) in ((s2wh, g1), (s2wh, g1l), (s2wl, g1)):
                            for cp in range(3):
                                nc.tensor.matmul(
                                    ps[:], wt[:, 128 * (3 * c + cp) : 128 * (3 * c + cp) + 128],
                                    dat[cp][:, 512 * s : 512 * s + 512],
                                    start=(k == 0), stop=(k == nmm - 1),
                                )
                                k += 1
                        if c == 0:
                            nc.tensor.matmul(ps[:], dcu[:], dcv[:], start=False, stop=True)
                        tb = sp.tile([128, 512], F32, tag="tq")
                        nc.vector.tensor_tensor(
                            tb[:].rearrange("p (a j) -> p a j", j=8),
                            ps[:].rearrange("p (a j) -> p a j", j=8),
                            rqv, op=mybir.AluOpType.mult,
                        )
                        nc.scalar.activation(tb[:], tb[:], COPY, bias=float(MAGIC))
                        nc.vector.scalar_tensor_tensor(
                            q[:, 512 * s : 512 * s + 512].rearrange("p (a j) -> p a j", j=8),
                            tb[:].rearrange("p (a j) -> p a j", j=8),
                            float(MAGIC),
                            qqv,
                            op0=mybir.AluOpType.subtract,
                            op1=mybir.AluOpType.mult,
                        )
            def s3(im, chans=(0, 1, 2)):
                qt_ = state[("q", im)]
                g3 = state.setdefault(("g3", im), [None, None, None])
                for c in chans:
                    g = g3p.tile([128, 2048], F32R, tag=f"g3_{c}")
                    g3[c] = g
                    for mt in range(4):
                        ps = psAp.tile([128, 512], F32, tag="psA")
                        for w in range(2):
                            for cc in range(2):
                                c2 = 2 * w + cc
                                nc.tensor.matmul(
                                    ps[:, 256 * w : 256 * w + 256],
                                    qt_[c][:, 512 * c2 + 128 * mt : 512 * c2 + 128 * mt + 128],
                                    s3r[:, 256 * cc : 256 * cc + 256],
                                    start=(cc == 0), stop=(cc == 1),
                                )
                        nc.scalar.activation(g[:, 512 * mt : 512 * mt + 512], ps[:], COPY)
            def s4(im):
                g3 = state[("g3", im)]
                for chan in range(3):
                    terms = _S4TERMS[chan]
                    for s in range(4):
                        ps = psBp.tile([128, 512], F32, tag="psB")
                        for ti, (csrc, _) in enumerate(terms):
                            wi = s4idx[(chan, csrc)]
                            nc.tensor.matmul(
                                ps[:], s4w[:, 128 * wi : 128 * wi + 128],
                                g3[csrc][:, 512 * s : 512 * s + 512],
                                start=(ti == 0), stop=False,
                            )
                        nc.tensor.matmul(ps[:], onu[:], onv[:], start=False, stop=True)
                        ot = op.tile([128, 512], F32, tag="ot")
                        nc.vector.tensor_scalar(
                            ot[:], ps[:], 0.0, 1.0,
                            op0=mybir.AluOpType.max, op1=mybir.AluOpType.min,
                        )
                        nc.sync.dma_start(
                            out_d[3 * im + chan, 128 * s : 128 * (s + 1), :], ot[:]
                        )

            # software-pipelined schedule over the two images
            s_load(0)
            s_load(1)
            s1(0)
            s2q(0)
            s1(1)
            s3(0)
            s4(0)
            s2q(1)
            s3(1)
            s4(1)
    nc.compile()
    return nc, hc


_COMPILED = None


def _get_compiled():
    global _COMPILED
    if _COMPILED is None:
        _COMPILED = _trace()
    return _COMPILED


def kernel(img, quality):
    img = np.ascontiguousarray(np.asarray(img, np.float32))
    quality = int(np.asarray(quality))
    nc, hc = _get_compiled()

    in_maps = []
    for core in range(N_CORES):
        rq, qq = _quant_inputs(quality, core)
        shard = np.ascontiguousarray(
            img[IMGS_PER_CORE * core : IMGS_PER_CORE * (core + 1)].reshape(SLICES, 512, 512)
        )
        in_maps.append({
            "img": shard, "rqpat": rq, "qpat": qq,
            "s1rhs": hc["s1rhs"], "s2wh": hc["s2wh"], "s2wl": hc["s2wl"], "s3rhs": hc["s3rhs"],
            "s4w": hc["s4w"], "dcu": hc["dcu"], "dcv": hc["dcv"],
            "onu": hc["onu"], "onv": hc["onv"],
        })

    res = run_bass_kernel_spmd(nc, in_maps, core_ids=list(range(N_CORES)))
    out = np.stack([res.results[c]["out"] for c in range(N_CORES)])
    return out.reshape(BS, 3, 512, 512)


if __name__ == "__main__":
    rng = np.random.default_rng(0)
    x = rng.random((BS, 3, 512, 512), dtype=np.float32)
    y = kernel(x, 80)
    print("kernel ran:", y.shape, y.dtype, float(y.min()), float(y.max()))
